# revision 14
# baseline (speedup 1.0000x reference)
"""KiloNeRF Trainium2 kernel: 4096 tiny MLPs, 512 points each, 8 NeuronCores.

Sharding: expert-parallel along the network axis (512 nets/core), 4 nets per
128-partition group (G=128 groups/core).

v5: fp8e4m3 DoubleRow matmuls, 7 PE instructions/group (the per-matmul
instruction floor, not cycles, dominates). Mixed precision (rel err
~1.6e-2 < 2e-2):
  L0  : W0 split hi/lo (host), pos single-fp8     -> 2 DR matmuls (chained)
  L1  : W1 split hi/lo (host), h1 single-fp8      -> 1 DR matmul (bcast rhs)
  Ld  : Wd_d 3-term + W_eff 1q, h2 single-fp8     -> 2 DR matmuls (chained);
        h2 is written into the xt tile so (dirh,h2)/(dirh,dirl) pair as
        DR k-tiles of single instructions
  Lout: rgb in bf16 (h3 bf16), alpha Wa split fp8 -> 1 bf16 + 1 DR (chained)
Feature layer folded into direction layer on host (W_eff = Wd_f@Wf). Output
bias (br/ba) applied on host. b0/b_eff folded into the L0/Ld matmuls as
hi/lo ones-row terms, so the two PSUM-capable engines carry minimal work:
ACT does relu->fp8/bf16 for h1/h3 pair-batched across 2 groups ([128,2,512]
psum tiles), DVE does h2 (bias port) and the output copy (2 groups of
outputs packed per [128,512] PSUM bank at 64-partition offsets; GPSIMD
cannot access PSUM on TRN2). DMAs batched; canvases scattered SBUF->SBUF.
"""

import sys

sys.path.insert(0, "/opt/trn_rl_repo")

import numpy as np
import ml_dtypes

N_NET = 4096
P = 512
PC = 63
DC = 27
H = 32
NCORES = 8
NPC = N_NET // NCORES  # nets per core = 512
NPG = 4  # nets per group (one 128-partition tile)
G = NPC // NPG  # groups per core = 128
B = 8  # groups per canvas-scatter batch
CK = 32  # groups per weight-preload chunk

BF16 = ml_dtypes.bfloat16
E4 = ml_dtypes.float8_e4m3

_nc_cache = {}


def _build_nc():
    import concourse.mybir as mybir
    import concourse.tile as tile
    from concourse import bacc

    nc = bacc.Bacc("TRN2")
    dt = mybir.dt
    AF = mybir.ActivationFunctionType
    ALU = mybir.AluOpType
    DR = mybir.MatmulPerfMode.DoubleRow

    with tile.TileContext(nc) as tc:
        xall_d = nc.dram_tensor("xall", [128, G, 4, P], dt.float8e4, kind="ExternalInput")
        w0f_d = nc.dram_tensor("w0f", [128, G, 4, 128], dt.float8e4, kind="ExternalInput")
        wdc_d = nc.dram_tensor("wdc", [4, 32, G, 6, 32], dt.float8e4, kind="ExternalInput")
        wr_d = nc.dram_tensor("wr", [128, G, 16], dt.bfloat16, kind="ExternalInput")
        wa_d = nc.dram_tensor("wa", [128, G, 2, 16], dt.float8e4, kind="ExternalInput")
        bia_d = nc.dram_tensor("bia", [128, G], dt.float32, kind="ExternalInput")
        out_d = nc.dram_tensor("out", [G // 2, 16, 2, P], dt.bfloat16, kind="ExternalOutput")

        with (
            tc.tile_pool(name="const", bufs=1) as constp,
            tc.tile_pool(name="cv", bufs=1) as cvp,
            tc.tile_pool(name="io", bufs=4) as io,
            tc.tile_pool(name="h1p", bufs=3) as h1p,
            tc.tile_pool(name="h3p", bufs=3) as h3p,
            tc.tile_pool(name="obp", bufs=2) as obp,
            tc.tile_pool(name="ps0", bufs=1, space="PSUM") as ps0,
            tc.tile_pool(name="ps1", bufs=2, space="PSUM") as ps1,
            tc.tile_pool(name="psd", bufs=1, space="PSUM") as psd,
            tc.tile_pool(name="pso", bufs=1, space="PSUM") as pso,
        ):
            # ---- persistent weight store (SBUF-resident) ----
            W0F = constp.tile([128, G, 4, 128], dt.float8e4, tag="W0F")
            WD = constp.tile([128, G, 6, 32], dt.float8e4, tag="WD")
            WR = constp.tile([128, G, 16], dt.bfloat16, tag="WR")
            WA = constp.tile([128, G, 2, 16], dt.float8e4, tag="WA")
            BIA = constp.tile([128, G], dt.float32, tag="BIA")
            nc.gpsimd.dma_start(out=BIA[:], in_=bia_d[:])
            nc.gpsimd.dma_start(out=WR[:], in_=wr_d[:])
            nc.gpsimd.dma_start(out=WA[:], in_=wa_d[:])

            def emit_preload_chunk(g0):
                nc.gpsimd.dma_start(
                    out=W0F[:, g0 : g0 + CK], in_=w0f_d[:, g0 : g0 + CK]
                )
                for j in range(4):
                    nc.gpsimd.dma_start(
                        out=WD[32 * j : 32 * j + 32, g0 : g0 + CK],
                        in_=wdc_d[j, :, g0 : g0 + CK],
                    )

            emit_preload_chunk(0)
            emit_preload_chunk(CK)

            # ---- canvas supertile ring for (W1h, W1l, Wdh, Wdl, Weh, Wel) ----
            cvs = []
            for i in range(4):
                cv = cvp.tile([128, B, 6, 128], dt.float8e4, tag=f"cv{i}", name=f"cv{i}")
                # zero off-block regions once; scatters only ever rewrite the
                # same block positions, so zeros persist across ring reuse.
                eng = nc.vector if i % 2 == 0 else nc.gpsimd
                flat = cv[:].rearrange("p a b c -> p (a b c)")
                for hcol in range(2):
                    eng.memset(flat[:, 3072 * hcol : 3072 * (hcol + 1)], 0.0)
                cvs.append(cv)

            def emit_scatter_batch(b):
                cv = cvs[b % 4]
                g0 = B * b
                for j in range(4):
                    nc.sync.dma_start(
                        out=cv[32 * j : 32 * j + 32, :, :, 32 * j : 32 * j + 32],
                        in_=WD[32 * j : 32 * j + 32, g0 : g0 + B],
                    )

            emit_scatter_batch(0)
            emit_scatter_batch(1)

            st = {}  # per-group live tiles

            def emit_xdma(g0):
                xt = io.tile([128, 4, 5, P], dt.float8e4, tag="xt")
                nc.sync.dma_start(out=xt[:, :, 0:4, :], in_=xall_d[:, g0 : g0 + 4])
                for q in range(4):
                    st.setdefault(g0 + q, {})["xt"] = xt
                    st[g0 + q]["xq"] = q

            def bcast2(ap_3d):
                return ap_3d.broadcast_to([128, 2, P])

            def emit_s0(g):
                s = st[g]
                xt, xq = s["xt"], s["xq"]
                h = g % 2
                if h == 0:
                    st["p0pair"] = ps0.tile([128, 2, P], dt.float32, tag="l0", name="l0")
                    st["h1pair"] = h1p.tile([128, 2, P], dt.float8e4, tag="h1", name="h1")
                p0 = st["p0pair"]
                nc.tensor.matmul(p0[:, h, :], lhsT=W0F[:, g, 0:2, :], rhs=xt[:, xq, 0:2, :], start=True, stop=False, perf_mode=DR)
                nc.tensor.matmul(p0[:, h, :], lhsT=W0F[:, g, 2:4, :], rhs=xt[:, xq, 0:2, :], start=False, stop=True, perf_mode=DR)
                s["h1pair"] = st["h1pair"]
                if h == 1:
                    # relu -> fp8 for both groups of the pair (b0 folded into L0)
                    nc.scalar.activation(st["h1pair"][:], p0[:], AF.Relu, scale=1.0)

            def emit_s1(g):
                s = st[g]
                cv = cvs[(g // B) % 4]
                q = g % B
                p1 = ps1.tile([128, P], dt.float32, tag="l1")
                h1b = bcast2(s["h1pair"][:, g % 2 : g % 2 + 1, :])
                nc.tensor.matmul(p1[:], lhsT=cv[:, q, 0:2, :], rhs=h1b, start=True, stop=True, perf_mode=DR)
                nc.vector.tensor_scalar(s["xt"][:, s["xq"], 4, :], p1[:], BIA[:, g : g + 1], 0.0, op0=ALU.add, op1=ALU.max)

            def emit_s2(g):
                s = st[g]
                cv = cvs[(g // B) % 4]
                q = g % B
                xt, xq = s["xt"], s["xq"]
                h = g % 2
                if h == 0:
                    st["pdpair"] = psd.tile([128, 2, P], dt.float32, tag="ld", name="ld")
                    st["h3pair"] = h3p.tile([128, 2, P], dt.bfloat16, tag="h3", name="h3")
                pd = st["pdpair"]
                # m4: Wdh*dirh + We*h2 (slots 2,4 of xt); m5: Wdl*dirh + Wdh*dirl
                nc.tensor.matmul(pd[:, h, :], lhsT=cv[:, q, 2:4, :], rhs=xt[:, xq, 2:5:2, :], start=True, stop=False, perf_mode=DR)
                nc.tensor.matmul(pd[:, h, :], lhsT=cv[:, q, 4:6, :], rhs=xt[:, xq, 2:4, :], start=False, stop=True, perf_mode=DR)
                s["h3pair"] = st["h3pair"]
                if h == 1:
                    # relu -> bf16 for both groups of the pair (b_eff folded into Ld)
                    nc.scalar.activation(st["h3pair"][:], pd[:], AF.Relu, scale=1.0)

            def emit_s3(g):
                s = st[g]
                q2 = g % 2
                if q2 == 0:
                    st["pso"] = pso.tile([16, 2, P], dt.float32, tag="lo", name="lo")
                po = st["pso"]
                sl = po[:, q2, :]
                xt, xq = s["xt"], s["xq"]
                h2b = bcast2(xt[:, xq, 4:5, :])
                nc.tensor.matmul(sl, lhsT=WR[:, g, :], rhs=s["h3pair"][:, q2, :], start=True, stop=False)
                nc.tensor.matmul(sl, lhsT=WA[:, g], rhs=h2b, start=False, stop=True, perf_mode=DR)
                if q2 == 1:
                    st["po_done"] = po
                del st[g]

            def emit_outcopy(g):
                # copy pair (g-1, g)'s packed outputs; emitted at the START of
                # the step after m8 so DVE services it before that step's h2
                # while the PE covers the pso-reuse gap with m1..m6.
                po = st.pop("po_done")
                ob = obp.tile([16, 2, P], dt.bfloat16, tag="ob")
                nc.vector.tensor_scalar_add(ob[:], po[:], 0.0)
                nc.sync.dma_start(out=out_d[g // 2], in_=ob[:])

            # ---- software-pipelined emission ----
            emit_xdma(0)
            emit_xdma(4)
            emit_xdma(8)
            for t in range(G + 8):
                if 0 <= t - 7 < G and (t - 7) % 2 == 1:
                    emit_outcopy(t - 7)
                if t % 4 == 0 and t + 12 < G:
                    emit_xdma(t + 12)
                if t < G:
                    emit_s0(t)
                if 0 <= t - 2 < G:
                    emit_s1(t - 2)
                if 0 <= t - 4 < G:
                    emit_s2(t - 4)
                if 0 <= t - 6 < G:
                    emit_s3(t - 6)
                if t % 8 == 0 and (t + 16) // 8 < G // B:
                    emit_scatter_batch((t + 16) // 8)
                if t % CK == 0 and (t + 2 * CK) < G:
                    emit_preload_chunk(t + 2 * CK)

    nc.compile()
    return nc


def _split8(a):
    hi = a.astype(E4)
    lo = (a - hi.astype(np.float32)).astype(E4)
    return hi, lo


def _pack_core(c, x, W0, b0, W1, b1, Wa, ba, Wf, bf, Wd, bd, Wr, br):
    lo_, hi_ = c * NPC, (c + 1) * NPC
    xT = np.ascontiguousarray(x[lo_:hi_].transpose(0, 2, 1))  # [512, 90, 512] f32

    # x: [128, G, 4, P]: slots pos0(nets01), pos1(nets23), dir_h, dir_l
    # pos0 row 63 = 1.0 (bias ones-row for b0); dir_h rows 32j+27 = 1.0 (b_eff)
    xarr = np.zeros((128, G, 4, P), dtype=E4)
    xa = xarr.transpose(1, 2, 0, 3)  # [G, 4, 128, P] view
    posr = xT[:, :PC, :].reshape(G, 4, PC, P)
    pos8 = posr.astype(E4)
    xa[:, 0, 0:PC] = pos8[:, 0]
    xa[:, 0, 64 : 64 + PC] = pos8[:, 1]
    xa[:, 1, 0:PC] = pos8[:, 2]
    xa[:, 1, 64 : 64 + PC] = pos8[:, 3]
    xa[:, 0, PC] = np.float32(1.0)
    dirr = xT[:, PC:, :].reshape(G, 4, DC, P)
    dh = dirr.astype(E4)
    dl = (dirr - dh.astype(np.float32)).astype(E4)
    for j in range(4):
        xa[:, 2, 32 * j : 32 * j + DC] = dh[:, j]
        xa[:, 2, 32 * j + DC] = np.float32(1.0)
        xa[:, 3, 32 * j : 32 * j + DC] = dl[:, j]

    # W0 canvases [128, G, 4, 128]: slots (W0h_p0, W0h_p1, W0l_p0, W0l_p1)
    # b0 hi/lo ones-rows at row 63 of the p0 canvases.
    w0T = W0[lo_:hi_].transpose(0, 2, 1).reshape(G, 4, PC, H)
    w0h, w0l = _split8(w0T)
    b0h, b0l = _split8(b0[lo_:hi_].reshape(G, 128))
    w0f = np.zeros((G, 2, 2, 128, 128), dtype=E4)  # [G, hl, pair, row, col]
    for hl, w8 in enumerate((w0h, w0l)):
        for j in range(4):
            pair, half = j // 2, j % 2
            w0f[:, hl, pair, 64 * half : 64 * half + PC, 32 * j : 32 * j + 32] = w8[:, j]
    w0f[:, 0, 0, PC, :] = b0h
    w0f[:, 1, 0, PC, :] = b0l
    w0f = np.ascontiguousarray(w0f.reshape(G, 4, 128, 128).transpose(2, 0, 1, 3))

    # fold feature layer into direction layer
    Wd_f = Wd[lo_:hi_, :, :H]
    Wd_d = Wd[lo_:hi_, :, H:]
    w_eff = np.matmul(Wd_f, Wf[lo_:hi_])
    b_eff = bd[lo_:hi_] + np.einsum("noi,ni->no", Wd_f, bf[lo_:hi_])

    # wdc [4, 32, G, 6, 32]: blocks (W1h, W1l, Wdh, Wdl, Weh, Wel)
    # b_eff hi/lo ones-rows at block row 27 of the Wdh/Wdl blocks.
    w1T = W1[lo_:hi_].transpose(0, 2, 1).reshape(G, 4, H, H)
    weT = w_eff.transpose(0, 2, 1).reshape(G, 4, H, H)
    wdT = Wd_d.transpose(0, 2, 1).reshape(G, 4, DC, H)
    w1h, w1l = _split8(w1T)
    weq = weT.astype(E4)
    wdh, wdl = _split8(wdT)
    beh, bel = _split8(b_eff.reshape(G, 4, H))
    wdc = np.zeros((G, 4, 32, 6, 32), dtype=E4)
    wdc[:, :, :, 0] = w1h
    wdc[:, :, :, 1] = w1l
    wdc[:, :, :DC, 2] = wdh
    wdc[:, :, DC, 2] = beh
    wdc[:, :, :, 3] = weq
    wdc[:, :, :DC, 4] = wdl
    wdc[:, :, DC, 4] = bel
    wdc[:, :, :DC, 5] = wdh
    wdc = np.ascontiguousarray(wdc.transpose(1, 2, 0, 3, 4))

    # wr [128, G, 64] bf16: col 4j+c (c<3) = Wr net j; wa [128, G, 2, 64] fp8
    wrT = Wr[lo_:hi_].transpose(0, 2, 1).reshape(G, 4, H, 3)
    waT = Wa[lo_:hi_].transpose(0, 2, 1).reshape(G, 4, H, 1)
    wah, wal = _split8(waT)
    wr = np.zeros((G, 128, 16), dtype=BF16)
    wa = np.zeros((G, 2, 128, 16), dtype=E4)
    for j in range(4):
        rows = slice(32 * j, 32 * j + 32)
        wr[:, rows, 4 * j : 4 * j + 3] = wrT[:, j].astype(BF16)
        wa[:, 0, rows, 4 * j + 3 : 4 * j + 4] = wah[:, j]
        wa[:, 1, rows, 4 * j + 3 : 4 * j + 4] = wal[:, j]
    wr = np.ascontiguousarray(wr.transpose(1, 0, 2))
    wa = np.ascontiguousarray(wa.transpose(2, 0, 1, 3))

    bia = np.ascontiguousarray(b1[lo_:hi_].reshape(G, 128).T.astype(np.float32))

    return {"xall": xarr, "w0f": w0f, "wdc": wdc, "wr": wr, "wa": wa, "bia": bia}


def kernel(**inputs):
    from concourse.bass_utils import run_bass_kernel_spmd

    if "nc" not in _nc_cache:
        _nc_cache["nc"] = _build_nc()
    nc = _nc_cache["nc"]

    from concurrent.futures import ThreadPoolExecutor

    with ThreadPoolExecutor(max_workers=8) as ex:
        in_maps = list(ex.map(lambda c: _pack_core(c, **inputs), range(NCORES)))

    res = run_bass_kernel_spmd(nc, in_maps, core_ids=list(range(NCORES)))

    br, ba = inputs["br"], inputs["ba"]
    out = np.empty((N_NET, P, 4), dtype=np.float32)
    for c in range(NCORES):
        o = res.results[c]["out"].astype(np.float32)  # [G//2, 16, 2, P]
        o = o.transpose(0, 2, 1, 3).reshape(NPC, 4, P)  # [pair, q, 16, P] -> nets
        out[c * NPC : (c + 1) * NPC] = o.transpose(0, 2, 1)
    out[:, :, 0:3] += br[:, None, :]
    out[:, :, 3:4] += ba[:, None, :]
    return out


# revision 15
# speedup vs baseline: 1.1072x; 1.1072x over previous
"""KiloNeRF Trainium2 kernel: 4096 tiny MLPs, 512 points each, 8 NeuronCores.

Sharding: expert-parallel along the network axis (512 nets/core), 4 nets per
128-partition group (G=128 groups/core).

v5: fp8e4m3 DoubleRow matmuls, 7 PE instructions/group (the per-matmul
instruction floor, not cycles, dominates). Mixed precision (rel err
~1.6e-2 < 2e-2):
  L0  : W0 split hi/lo (host), pos single-fp8     -> 2 DR matmuls (chained)
  L1  : W1 split hi/lo (host), h1 single-fp8      -> 1 DR matmul (bcast rhs)
  Ld  : Wd_d 3-term + W_eff 1q, h2 single-fp8     -> 2 DR matmuls (chained);
        h2 is written into the xt tile so (dirh,h2)/(dirh,dirl) pair as
        DR k-tiles of single instructions
  Lout: rgb in bf16 (h3 bf16), alpha Wa split fp8 -> 1 bf16 + 1 DR (chained)
Feature layer folded into direction layer on host (W_eff = Wd_f@Wf). Output
bias (br/ba) applied on host. b0/b_eff folded into the L0/Ld matmuls as
hi/lo ones-row terms, so the two PSUM-capable engines carry minimal work:
ACT does relu->fp8/bf16 for h1/h3 pair-batched across 2 groups ([128,2,512]
psum tiles), DVE does h2 (bias port) and the output copy (2 groups of
outputs packed per [128,512] PSUM bank at 64-partition offsets; GPSIMD
cannot access PSUM on TRN2). DMAs batched; canvases scattered SBUF->SBUF.
"""

import sys

sys.path.insert(0, "/opt/trn_rl_repo")

import numpy as np
import ml_dtypes

N_NET = 4096
P = 512
PC = 63
DC = 27
H = 32
NCORES = 8
NPC = N_NET // NCORES  # nets per core = 512
NPG = 4  # nets per group (one 128-partition tile)
G = NPC // NPG  # groups per core = 128
B = 8  # groups per canvas-scatter batch
CK = 32  # groups per weight-preload chunk

BF16 = ml_dtypes.bfloat16
E4 = ml_dtypes.float8_e4m3

_nc_cache = {}


def _build_nc():
    import concourse.mybir as mybir
    import concourse.tile as tile
    from concourse import bacc

    nc = bacc.Bacc("TRN2")
    dt = mybir.dt
    AF = mybir.ActivationFunctionType
    ALU = mybir.AluOpType
    DR = mybir.MatmulPerfMode.DoubleRow

    with tile.TileContext(nc) as tc:
        xall_d = nc.dram_tensor("xall", [128, G, 4, P], dt.float8e4, kind="ExternalInput")
        w0f_d = nc.dram_tensor("w0f", [128, G, 4, 128], dt.float8e4, kind="ExternalInput")
        wdc_d = nc.dram_tensor("wdc", [4, 32, G, 6, 32], dt.float8e4, kind="ExternalInput")
        wr_d = nc.dram_tensor("wr", [128, G, 16], dt.bfloat16, kind="ExternalInput")
        wa_d = nc.dram_tensor("wa", [128, G, 2, 16], dt.float8e4, kind="ExternalInput")
        bia_d = nc.dram_tensor("bia", [128, G], dt.float32, kind="ExternalInput")
        out_d = nc.dram_tensor("out", [G // 2, 16, 2, P], dt.bfloat16, kind="ExternalOutput")

        with (
            tc.tile_pool(name="const", bufs=1) as constp,
            tc.tile_pool(name="cv", bufs=1) as cvp,
            tc.tile_pool(name="io", bufs=4) as io,
            tc.tile_pool(name="h1p", bufs=4) as h1p,
            tc.tile_pool(name="h3p", bufs=4) as h3p,
            tc.tile_pool(name="obp", bufs=2) as obp,
            tc.tile_pool(name="ps0", bufs=2, space="PSUM") as ps0,
            tc.tile_pool(name="ps1", bufs=2, space="PSUM") as ps1,
            tc.tile_pool(name="psd", bufs=2, space="PSUM") as psd,
            tc.tile_pool(name="pso", bufs=2, space="PSUM") as pso,
        ):
            # ---- persistent weight store (SBUF-resident) ----
            W0F = constp.tile([128, G, 4, 128], dt.float8e4, tag="W0F")
            WD = constp.tile([128, G, 6, 32], dt.float8e4, tag="WD")
            WR = constp.tile([128, G, 16], dt.bfloat16, tag="WR")
            WA = constp.tile([128, G, 2, 16], dt.float8e4, tag="WA")
            BIA = constp.tile([128, G], dt.float32, tag="BIA")
            nc.gpsimd.dma_start(out=BIA[:], in_=bia_d[:])
            nc.gpsimd.dma_start(out=WR[:], in_=wr_d[:])
            nc.gpsimd.dma_start(out=WA[:], in_=wa_d[:])

            def emit_preload_chunk(g0):
                nc.gpsimd.dma_start(
                    out=W0F[:, g0 : g0 + CK], in_=w0f_d[:, g0 : g0 + CK]
                )
                for j in range(4):
                    nc.gpsimd.dma_start(
                        out=WD[32 * j : 32 * j + 32, g0 : g0 + CK],
                        in_=wdc_d[j, :, g0 : g0 + CK],
                    )

            emit_preload_chunk(0)
            emit_preload_chunk(CK)

            # ---- canvas supertile ring for (W1h, W1l, Wdh, Wdl, Weh, Wel) ----
            cvs = []
            for i in range(4):
                cv = cvp.tile([128, B, 6, 128], dt.float8e4, tag=f"cv{i}", name=f"cv{i}")
                # zero off-block regions once; scatters only ever rewrite the
                # same block positions, so zeros persist across ring reuse.
                eng = nc.vector if i % 2 == 0 else nc.gpsimd
                flat = cv[:].rearrange("p a b c -> p (a b c)")
                for hcol in range(2):
                    eng.memset(flat[:, 3072 * hcol : 3072 * (hcol + 1)], 0.0)
                cvs.append(cv)

            def emit_scatter_batch(b):
                cv = cvs[b % 4]
                g0 = B * b
                for j in range(4):
                    nc.sync.dma_start(
                        out=cv[32 * j : 32 * j + 32, :, :, 32 * j : 32 * j + 32],
                        in_=WD[32 * j : 32 * j + 32, g0 : g0 + B],
                    )

            emit_scatter_batch(0)
            emit_scatter_batch(1)

            st = {}  # per-group live tiles

            def emit_xdma(g0):
                xt = io.tile([128, 4, 5, P], dt.float8e4, tag="xt")
                nc.sync.dma_start(out=xt[:, :, 0:4, :], in_=xall_d[:, g0 : g0 + 4])
                for q in range(4):
                    st.setdefault(g0 + q, {})["xt"] = xt
                    st[g0 + q]["xq"] = q

            def bcast2(ap_3d):
                return ap_3d.broadcast_to([128, 2, P])

            def emit_s0(g):
                s = st[g]
                xt, xq = s["xt"], s["xq"]
                p0 = ps0.tile([128, P], dt.float32, tag="l0")
                nc.tensor.matmul(p0[:], lhsT=W0F[:, g, 0:2, :], rhs=xt[:, xq, 0:2, :], start=True, stop=False, perf_mode=DR)
                nc.tensor.matmul(p0[:], lhsT=W0F[:, g, 2:4, :], rhs=xt[:, xq, 0:2, :], start=False, stop=True, perf_mode=DR)
                h1 = h1p.tile([128, P], dt.float8e4, tag="h1")
                # b0 folded into L0 ones-row; relu only
                nc.vector.tensor_scalar(h1[:], p0[:], 0.0, 0.0, op0=ALU.add, op1=ALU.max)
                s["h1"] = h1

            def emit_s1(g):
                s = st[g]
                cv = cvs[(g // B) % 4]
                q = g % B
                p1 = ps1.tile([128, P], dt.float32, tag="l1")
                h1b = bcast2(s["h1"][:].rearrange("p (o f) -> p o f", o=1))
                nc.tensor.matmul(p1[:], lhsT=cv[:, q, 0:2, :], rhs=h1b, start=True, stop=True, perf_mode=DR)
                nc.vector.tensor_scalar(s["xt"][:, s["xq"], 4, :], p1[:], BIA[:, g : g + 1], 0.0, op0=ALU.add, op1=ALU.max)

            def emit_s2(g):
                s = st[g]
                cv = cvs[(g // B) % 4]
                q = g % B
                xt, xq = s["xt"], s["xq"]
                pd = psd.tile([128, P], dt.float32, tag="ld")
                # m4: Wdh*dirh + We*h2 (slots 2,4 of xt); m5: Wdl*dirh + Wdh*dirl
                nc.tensor.matmul(pd[:], lhsT=cv[:, q, 2:4, :], rhs=xt[:, xq, 2:5:2, :], start=True, stop=False, perf_mode=DR)
                nc.tensor.matmul(pd[:], lhsT=cv[:, q, 4:6, :], rhs=xt[:, xq, 2:4, :], start=False, stop=True, perf_mode=DR)
                h3 = h3p.tile([128, P], dt.bfloat16, tag="h3")
                # b_eff folded into Ld ones-row; relu only
                nc.scalar.activation(h3[:], pd[:], AF.Relu, scale=1.0)
                s["h3"] = h3

            def emit_s3(g):
                s = st[g]
                q2 = g % 2
                if q2 == 0:
                    st["ob"] = obp.tile([16, 2, P], dt.bfloat16, tag="ob", name="ob")
                po = pso.tile([16, P], dt.float32, tag="lo")
                xt, xq = s["xt"], s["xq"]
                h2b = bcast2(xt[:, xq, 4:5, :])
                nc.tensor.matmul(po[:], lhsT=WR[:, g, :], rhs=s["h3"][:], start=True, stop=False)
                nc.tensor.matmul(po[:], lhsT=WA[:, g], rhs=h2b, start=False, stop=True, perf_mode=DR)
                ob = st["ob"]
                nc.scalar.activation(ob[:, q2, :], po[:], AF.Copy)
                if q2 == 1:
                    nc.sync.dma_start(out=out_d[g // 2], in_=ob[:])
                del st[g]

            # ---- software-pipelined emission ----
            emit_xdma(0)
            emit_xdma(4)
            emit_xdma(8)
            for t in range(G + 8):
                if t % 4 == 0 and t + 12 < G:
                    emit_xdma(t + 12)
                if t < G:
                    emit_s0(t)
                if 0 <= t - 2 < G:
                    emit_s1(t - 2)
                if 0 <= t - 4 < G:
                    emit_s2(t - 4)
                if 0 <= t - 6 < G:
                    emit_s3(t - 6)
                if t % 8 == 0 and (t + 16) // 8 < G // B:
                    emit_scatter_batch((t + 16) // 8)
                if t % CK == 0 and (t + 2 * CK) < G:
                    emit_preload_chunk(t + 2 * CK)

    nc.compile()
    return nc


def _split8(a):
    hi = a.astype(E4)
    lo = (a - hi.astype(np.float32)).astype(E4)
    return hi, lo


def _pack_core(c, x, W0, b0, W1, b1, Wa, ba, Wf, bf, Wd, bd, Wr, br):
    lo_, hi_ = c * NPC, (c + 1) * NPC
    xT = np.ascontiguousarray(x[lo_:hi_].transpose(0, 2, 1))  # [512, 90, 512] f32

    # x: [128, G, 4, P]: slots pos0(nets01), pos1(nets23), dir_h, dir_l
    # pos0 row 63 = 1.0 (bias ones-row for b0); dir_h rows 32j+27 = 1.0 (b_eff)
    xarr = np.zeros((128, G, 4, P), dtype=E4)
    xa = xarr.transpose(1, 2, 0, 3)  # [G, 4, 128, P] view
    posr = xT[:, :PC, :].reshape(G, 4, PC, P)
    pos8 = posr.astype(E4)
    xa[:, 0, 0:PC] = pos8[:, 0]
    xa[:, 0, 64 : 64 + PC] = pos8[:, 1]
    xa[:, 1, 0:PC] = pos8[:, 2]
    xa[:, 1, 64 : 64 + PC] = pos8[:, 3]
    xa[:, 0, PC] = np.float32(1.0)
    dirr = xT[:, PC:, :].reshape(G, 4, DC, P)
    dh = dirr.astype(E4)
    dl = (dirr - dh.astype(np.float32)).astype(E4)
    for j in range(4):
        xa[:, 2, 32 * j : 32 * j + DC] = dh[:, j]
        xa[:, 2, 32 * j + DC] = np.float32(1.0)
        xa[:, 3, 32 * j : 32 * j + DC] = dl[:, j]

    # W0 canvases [128, G, 4, 128]: slots (W0h_p0, W0h_p1, W0l_p0, W0l_p1)
    # b0 hi/lo ones-rows at row 63 of the p0 canvases.
    w0T = W0[lo_:hi_].transpose(0, 2, 1).reshape(G, 4, PC, H)
    w0h, w0l = _split8(w0T)
    b0h, b0l = _split8(b0[lo_:hi_].reshape(G, 128))
    w0f = np.zeros((G, 2, 2, 128, 128), dtype=E4)  # [G, hl, pair, row, col]
    for hl, w8 in enumerate((w0h, w0l)):
        for j in range(4):
            pair, half = j // 2, j % 2
            w0f[:, hl, pair, 64 * half : 64 * half + PC, 32 * j : 32 * j + 32] = w8[:, j]
    w0f[:, 0, 0, PC, :] = b0h
    w0f[:, 1, 0, PC, :] = b0l
    w0f = np.ascontiguousarray(w0f.reshape(G, 4, 128, 128).transpose(2, 0, 1, 3))

    # fold feature layer into direction layer
    Wd_f = Wd[lo_:hi_, :, :H]
    Wd_d = Wd[lo_:hi_, :, H:]
    w_eff = np.matmul(Wd_f, Wf[lo_:hi_])
    b_eff = bd[lo_:hi_] + np.einsum("noi,ni->no", Wd_f, bf[lo_:hi_])

    # wdc [4, 32, G, 6, 32]: blocks (W1h, W1l, Wdh, Wdl, Weh, Wel)
    # b_eff hi/lo ones-rows at block row 27 of the Wdh/Wdl blocks.
    w1T = W1[lo_:hi_].transpose(0, 2, 1).reshape(G, 4, H, H)
    weT = w_eff.transpose(0, 2, 1).reshape(G, 4, H, H)
    wdT = Wd_d.transpose(0, 2, 1).reshape(G, 4, DC, H)
    w1h, w1l = _split8(w1T)
    weq = weT.astype(E4)
    wdh, wdl = _split8(wdT)
    beh, bel = _split8(b_eff.reshape(G, 4, H))
    wdc = np.zeros((G, 4, 32, 6, 32), dtype=E4)
    wdc[:, :, :, 0] = w1h
    wdc[:, :, :, 1] = w1l
    wdc[:, :, :DC, 2] = wdh
    wdc[:, :, DC, 2] = beh
    wdc[:, :, :, 3] = weq
    wdc[:, :, :DC, 4] = wdl
    wdc[:, :, DC, 4] = bel
    wdc[:, :, :DC, 5] = wdh
    wdc = np.ascontiguousarray(wdc.transpose(1, 2, 0, 3, 4))

    # wr [128, G, 64] bf16: col 4j+c (c<3) = Wr net j; wa [128, G, 2, 64] fp8
    wrT = Wr[lo_:hi_].transpose(0, 2, 1).reshape(G, 4, H, 3)
    waT = Wa[lo_:hi_].transpose(0, 2, 1).reshape(G, 4, H, 1)
    wah, wal = _split8(waT)
    wr = np.zeros((G, 128, 16), dtype=BF16)
    wa = np.zeros((G, 2, 128, 16), dtype=E4)
    for j in range(4):
        rows = slice(32 * j, 32 * j + 32)
        wr[:, rows, 4 * j : 4 * j + 3] = wrT[:, j].astype(BF16)
        wa[:, 0, rows, 4 * j + 3 : 4 * j + 4] = wah[:, j]
        wa[:, 1, rows, 4 * j + 3 : 4 * j + 4] = wal[:, j]
    wr = np.ascontiguousarray(wr.transpose(1, 0, 2))
    wa = np.ascontiguousarray(wa.transpose(2, 0, 1, 3))

    bia = np.ascontiguousarray(b1[lo_:hi_].reshape(G, 128).T.astype(np.float32))

    return {"xall": xarr, "w0f": w0f, "wdc": wdc, "wr": wr, "wa": wa, "bia": bia}


def kernel(**inputs):
    from concourse.bass_utils import run_bass_kernel_spmd

    if "nc" not in _nc_cache:
        _nc_cache["nc"] = _build_nc()
    nc = _nc_cache["nc"]

    from concurrent.futures import ThreadPoolExecutor

    with ThreadPoolExecutor(max_workers=8) as ex:
        in_maps = list(ex.map(lambda c: _pack_core(c, **inputs), range(NCORES)))

    res = run_bass_kernel_spmd(nc, in_maps, core_ids=list(range(NCORES)))

    br, ba = inputs["br"], inputs["ba"]
    out = np.empty((N_NET, P, 4), dtype=np.float32)
    for c in range(NCORES):
        o = res.results[c]["out"].astype(np.float32)  # [G//2, 16, 2, P]
        o = o.transpose(0, 2, 1, 3).reshape(NPC, 4, P)  # [pair, q, 16, P] -> nets
        out[c * NPC : (c + 1) * NPC] = o.transpose(0, 2, 1)
    out[:, :, 0:3] += br[:, None, :]
    out[:, :, 3:4] += ba[:, None, :]
    return out


# revision 16
# speedup vs baseline: 1.2019x; 1.0855x over previous
"""KiloNeRF Trainium2 kernel: 4096 tiny MLPs, 512 points each, 8 NeuronCores.

Sharding: expert-parallel along the network axis (512 nets/core), 4 nets per
128-partition group (G=128 groups/core).

v7: instruction-minimal hybrid. The PE has a ~215ns per-matmul issue floor,
so instruction count (not cycles) dominates; fp8 DoubleRow is used only
where it merges two matmuls into one:
  m1: L0 via one fp8e4m3 DR matmul (two pos k-tiles, W0 single-quantized)
  m3: L1 bf16      m4: Ld-dir bf16   m5: Ld-h2 bf16 (accum with m4)
  m6: rgb bf16     m7: alpha bf16 (accum with m6)
= 6 matmuls/group, rel err ~1.1e-2 < 2e-2. Feature layer folded into the
direction layer on host (W_eff = Wd_f@Wf); output bias on host.
Aux ops (only ACT+DVE reach PSUM on TRN2): DVE h1, h2 relu+bias; ACT h3
relu+bias and the [16,2,512]-packed output copy. Each aux op is emitted one
step after its producing matmul so engines start steps with ready inputs.
All psum pools double-buffered (no serial psum-reuse chains). DMAs batched
on sync; weight canvases scattered SBUF->SBUF from compact blocks.
"""

import sys

sys.path.insert(0, "/opt/trn_rl_repo")

import numpy as np
import ml_dtypes

N_NET = 4096
P = 512
PC = 63
DC = 27
H = 32
NCORES = 8
NPC = N_NET // NCORES  # nets per core = 512
NPG = 4  # nets per group (one 128-partition tile)
G = NPC // NPG  # groups per core = 128
B = 8  # groups per canvas-scatter batch
CK = 32  # groups per weight-preload chunk

BF16 = ml_dtypes.bfloat16
E4 = ml_dtypes.float8_e4m3

_nc_cache = {}


def _build_nc():
    import concourse.mybir as mybir
    import concourse.tile as tile
    from concourse import bacc

    nc = bacc.Bacc("TRN2")
    dt = mybir.dt
    AF = mybir.ActivationFunctionType
    ALU = mybir.AluOpType
    DR = mybir.MatmulPerfMode.DoubleRow

    with tile.TileContext(nc) as tc:
        xall_d = nc.dram_tensor("xall", [128, G, 2, P], dt.float8e4, kind="ExternalInput")
        xdir_d = nc.dram_tensor("xdir", [128, G, P], dt.bfloat16, kind="ExternalInput")
        w0f_d = nc.dram_tensor("w0f", [128, G, 2, 128], dt.float8e4, kind="ExternalInput")
        wdc_d = nc.dram_tensor("wdc", [4, 32, G, 3, 32], dt.bfloat16, kind="ExternalInput")
        wr_d = nc.dram_tensor("wr", [128, G, 16], dt.bfloat16, kind="ExternalInput")
        wa_d = nc.dram_tensor("wa", [128, G, 16], dt.bfloat16, kind="ExternalInput")
        bia_d = nc.dram_tensor("bia", [128, G, 3], dt.float32, kind="ExternalInput")
        out_d = nc.dram_tensor("out", [G // 2, 16, 2, P], dt.bfloat16, kind="ExternalOutput")

        with (
            tc.tile_pool(name="const", bufs=1) as constp,
            tc.tile_pool(name="cv", bufs=1) as cvp,
            tc.tile_pool(name="io", bufs=4) as io,
            tc.tile_pool(name="iod", bufs=4) as iod,
            tc.tile_pool(name="h1p", bufs=4) as h1p,
            tc.tile_pool(name="h2p", bufs=6) as h2p,
            tc.tile_pool(name="h3p", bufs=4) as h3p,
            tc.tile_pool(name="obp", bufs=2) as obp,
            tc.tile_pool(name="ps0", bufs=2, space="PSUM") as ps0,
            tc.tile_pool(name="ps1", bufs=2, space="PSUM") as ps1,
            tc.tile_pool(name="psd", bufs=2, space="PSUM") as psd,
            tc.tile_pool(name="pso", bufs=2, space="PSUM") as pso,
        ):
            # ---- persistent weight store (SBUF-resident) ----
            W0F = constp.tile([128, G, 2, 128], dt.float8e4, tag="W0F")
            WD = constp.tile([128, G, 3, 32], dt.bfloat16, tag="WD")
            WR = constp.tile([128, G, 16], dt.bfloat16, tag="WR")
            WA = constp.tile([128, G, 16], dt.bfloat16, tag="WA")
            BIA = constp.tile([128, G, 3], dt.float32, tag="BIA")
            nc.gpsimd.dma_start(out=BIA[:], in_=bia_d[:])
            nc.gpsimd.dma_start(out=WR[:], in_=wr_d[:])
            nc.gpsimd.dma_start(out=WA[:], in_=wa_d[:])

            def emit_preload_chunk(g0):
                nc.gpsimd.dma_start(
                    out=W0F[:, g0 : g0 + CK], in_=w0f_d[:, g0 : g0 + CK]
                )
                for j in range(4):
                    nc.gpsimd.dma_start(
                        out=WD[32 * j : 32 * j + 32, g0 : g0 + CK],
                        in_=wdc_d[j, :, g0 : g0 + CK],
                    )

            emit_preload_chunk(0)
            emit_preload_chunk(CK)

            # ---- canvas supertile ring for (W1, Wdd, We) ----
            cvs = []
            for i in range(4):
                cv = cvp.tile([128, B, 3, 128], dt.bfloat16, tag=f"cv{i}", name=f"cv{i}")
                # zero off-block regions once; scatters only ever rewrite the
                # same block positions, so zeros persist across ring reuse.
                eng = nc.vector if i % 2 == 0 else nc.gpsimd
                flat = cv[:].rearrange("p a b c -> p (a b c)")
                for hcol in range(2):
                    eng.memset(flat[:, 1536 * hcol : 1536 * (hcol + 1)], 0.0)
                cvs.append(cv)

            def emit_scatter_batch(b):
                cv = cvs[b % 4]
                g0 = B * b
                for j in range(4):
                    nc.sync.dma_start(
                        out=cv[32 * j : 32 * j + 32, :, :, 32 * j : 32 * j + 32],
                        in_=WD[32 * j : 32 * j + 32, g0 : g0 + B],
                    )

            emit_scatter_batch(0)
            emit_scatter_batch(1)

            st = {}  # per-group live tiles

            def emit_xdma(g0):
                xt = io.tile([128, 4, 2, P], dt.float8e4, tag="xt")
                nc.sync.dma_start(out=xt[:], in_=xall_d[:, g0 : g0 + 4])
                xd = iod.tile([128, 4, P], dt.bfloat16, tag="xd")
                nc.sync.dma_start(out=xd[:], in_=xdir_d[:, g0 : g0 + 4])
                for q in range(4):
                    st.setdefault(g0 + q, {})["xt"] = xt
                    st[g0 + q]["xd"] = xd
                    st[g0 + q]["xq"] = q

            def emit_s0(g):
                s = st[g]
                p0 = ps0.tile([128, P], dt.float32, tag="l0")
                nc.tensor.matmul(p0[:], lhsT=W0F[:, g], rhs=s["xt"][:, s["xq"]], start=True, stop=True, perf_mode=DR)
                s["p0"] = p0

            def emit_h1(g):
                s = st[g]
                h1 = h1p.tile([128, P], dt.bfloat16, tag="h1")
                nc.vector.tensor_scalar(h1[:], s.pop("p0")[:], BIA[:, g, 0:1], 0.0, op0=ALU.add, op1=ALU.max)
                s["h1"] = h1

            def emit_s1(g):
                s = st[g]
                cv = cvs[(g // B) % 4]
                p1 = ps1.tile([128, P], dt.float32, tag="l1")
                nc.tensor.matmul(p1[:], lhsT=cv[:, g % B, 0, :], rhs=s.pop("h1")[:], start=True, stop=True)
                s["p1"] = p1

            def emit_h2(g):
                s = st[g]
                h2 = h2p.tile([128, P], dt.bfloat16, tag="h2")
                nc.vector.tensor_scalar(h2[:], s.pop("p1")[:], BIA[:, g, 1:2], 0.0, op0=ALU.add, op1=ALU.max)
                s["h2"] = h2

            def emit_s2(g):
                s = st[g]
                cv = cvs[(g // B) % 4]
                q = g % B
                pd = psd.tile([128, P], dt.float32, tag="ld")
                nc.tensor.matmul(pd[:], lhsT=cv[:, q, 1, :], rhs=s["xd"][:, s["xq"], :], start=True, stop=False)
                nc.tensor.matmul(pd[:], lhsT=cv[:, q, 2, :], rhs=s["h2"][:], start=False, stop=True)
                s["pd"] = pd

            def emit_h3(g):
                s = st[g]
                h3 = h3p.tile([128, P], dt.bfloat16, tag="h3")
                nc.scalar.activation(h3[:], s.pop("pd")[:], AF.Relu, bias=BIA[:, g, 2:3], scale=1.0)
                s["h3"] = h3

            def emit_s3(g):
                s = st[g]
                po = pso.tile([16, P], dt.float32, tag="lo")
                nc.tensor.matmul(po[:], lhsT=WR[:, g, :], rhs=s.pop("h3")[:], start=True, stop=False)
                nc.tensor.matmul(po[:], lhsT=WA[:, g, :], rhs=s.pop("h2")[:], start=False, stop=True)
                s["po"] = po

            def emit_outcopy(g):
                s = st.pop(g)
                q2 = g % 2
                if q2 == 0:
                    st["ob"] = obp.tile([16, 2, P], dt.bfloat16, tag="ob", name="ob")
                ob = st["ob"]
                nc.scalar.activation(ob[:, q2, :], s.pop("po")[:], AF.Copy)
                if q2 == 1:
                    nc.sync.dma_start(out=out_d[g // 2], in_=ob[:])

            # ---- software-pipelined emission; aux ops lag their producer
            # matmul by one step so engines start each step with ready work ----
            emit_xdma(0)
            emit_xdma(4)
            emit_xdma(8)
            for t in range(G + 8):
                if 0 <= t - 1 < G:
                    emit_h1(t - 1)
                if 0 <= t - 3 < G:
                    emit_h2(t - 3)
                if 0 <= t - 5 < G:
                    emit_h3(t - 5)
                if 0 <= t - 7 < G:
                    emit_outcopy(t - 7)
                if t % 4 == 0 and t + 12 < G:
                    emit_xdma(t + 12)
                if t < G:
                    emit_s0(t)
                if 0 <= t - 2 < G:
                    emit_s1(t - 2)
                if 0 <= t - 4 < G:
                    emit_s2(t - 4)
                if 0 <= t - 6 < G:
                    emit_s3(t - 6)
                if t % 8 == 0 and (t + 16) // 8 < G // B:
                    emit_scatter_batch((t + 16) // 8)
                if t % CK == 0 and (t + 2 * CK) < G:
                    emit_preload_chunk(t + 2 * CK)

    nc.compile()
    return nc


def _pack_core(c, x, W0, b0, W1, b1, Wa, ba, Wf, bf, Wd, bd, Wr, br):
    lo_, hi_ = c * NPC, (c + 1) * NPC
    xT = np.ascontiguousarray(x[lo_:hi_].transpose(0, 2, 1))  # [512, 90, 512] f32

    # pos canvases: [128, G, 2, P] fp8: slot0 nets01, slot1 nets23
    xarr = np.zeros((128, G, 2, P), dtype=E4)
    xa = xarr.transpose(1, 2, 0, 3)  # [G, 2, 128, P] view
    posr = xT[:, :PC, :].reshape(G, 4, PC, P)
    pos8 = posr.astype(E4)
    xa[:, 0, 0:PC] = pos8[:, 0]
    xa[:, 0, 64 : 64 + PC] = pos8[:, 1]
    xa[:, 1, 0:PC] = pos8[:, 2]
    xa[:, 1, 64 : 64 + PC] = pos8[:, 3]

    # dir canvas: [128, G, P] bf16: rows 32j..32j+27 = net j dir channels
    xdir = np.zeros((128, G, P), dtype=BF16)
    xd = xdir.transpose(1, 0, 2)
    dirr = xT[:, PC:, :].reshape(G, 4, DC, P).astype(BF16)
    for j in range(4):
        xd[:, 32 * j : 32 * j + DC] = dirr[:, j]

    # W0 canvases [128, G, 2, 128] fp8, single-quantized
    w0T = W0[lo_:hi_].transpose(0, 2, 1).reshape(G, 4, PC, H).astype(E4)
    w0f = np.zeros((G, 2, 128, 128), dtype=E4)  # [G, pair, row, col]
    for j in range(4):
        pair, half = j // 2, j % 2
        w0f[:, pair, 64 * half : 64 * half + PC, 32 * j : 32 * j + 32] = w0T[:, j]
    w0f = np.ascontiguousarray(w0f.transpose(2, 0, 1, 3))

    # fold feature layer into direction layer
    Wd_f = Wd[lo_:hi_, :, :H]
    Wd_d = Wd[lo_:hi_, :, H:]
    w_eff = np.matmul(Wd_f, Wf[lo_:hi_])
    b_eff = bd[lo_:hi_] + np.einsum("noi,ni->no", Wd_f, bf[lo_:hi_])

    # wdc [4, 32, G, 3, 32] bf16: blocks (W1, Wdd, We)
    w1T = W1[lo_:hi_].transpose(0, 2, 1).reshape(G, 4, H, H).astype(BF16)
    weT = w_eff.transpose(0, 2, 1).reshape(G, 4, H, H).astype(BF16)
    wdT = Wd_d.transpose(0, 2, 1).reshape(G, 4, DC, H).astype(BF16)
    wdc = np.zeros((G, 4, 32, 3, 32), dtype=BF16)
    wdc[:, :, :, 0] = w1T
    wdc[:, :, :DC, 1] = wdT
    wdc[:, :, :, 2] = weT
    wdc = np.ascontiguousarray(wdc.transpose(1, 2, 0, 3, 4))

    # wr/wa [128, G, 16] bf16: col 4j+c = output c of net j (c=3 -> alpha)
    wrT = Wr[lo_:hi_].transpose(0, 2, 1).reshape(G, 4, H, 3).astype(BF16)
    waT = Wa[lo_:hi_].transpose(0, 2, 1).reshape(G, 4, H, 1).astype(BF16)
    wr = np.zeros((G, 128, 16), dtype=BF16)
    wa = np.zeros((G, 128, 16), dtype=BF16)
    for j in range(4):
        rows = slice(32 * j, 32 * j + 32)
        wr[:, rows, 4 * j : 4 * j + 3] = wrT[:, j]
        wa[:, rows, 4 * j + 3 : 4 * j + 4] = waT[:, j]
    wr = np.ascontiguousarray(wr.transpose(1, 0, 2))
    wa = np.ascontiguousarray(wa.transpose(1, 0, 2))

    bias = np.zeros((G, 128, 3), dtype=np.float32)
    bias[:, :, 0] = b0[lo_:hi_].reshape(G, 128)
    bias[:, :, 1] = b1[lo_:hi_].reshape(G, 128)
    bias[:, :, 2] = b_eff.reshape(G, 128)
    bia = np.ascontiguousarray(bias.transpose(1, 0, 2))

    return {
        "xall": xarr,
        "xdir": xdir,
        "w0f": w0f,
        "wdc": wdc,
        "wr": wr,
        "wa": wa,
        "bia": bia,
    }


def kernel(**inputs):
    from concourse.bass_utils import run_bass_kernel_spmd

    if "nc" not in _nc_cache:
        _nc_cache["nc"] = _build_nc()
    nc = _nc_cache["nc"]

    from concurrent.futures import ThreadPoolExecutor

    with ThreadPoolExecutor(max_workers=8) as ex:
        in_maps = list(ex.map(lambda c: _pack_core(c, **inputs), range(NCORES)))

    res = run_bass_kernel_spmd(nc, in_maps, core_ids=list(range(NCORES)))

    br, ba = inputs["br"], inputs["ba"]
    out = np.empty((N_NET, P, 4), dtype=np.float32)
    for c in range(NCORES):
        o = res.results[c]["out"].astype(np.float32)  # [G//2, 16, 2, P]
        o = o.transpose(0, 2, 1, 3).reshape(NPC, 4, P)  # [pair, q, 16, P] -> nets
        out[c * NPC : (c + 1) * NPC] = o.transpose(0, 2, 1)
    out[:, :, 0:3] += br[:, None, :]
    out[:, :, 3:4] += ba[:, None, :]
    return out
